# revision 43
# baseline (speedup 1.0000x reference)
"""Trainium2 Bass kernel for Attn_PointLevel (sparse_attention).

Math (per (b,v,p) patch, L=48 tokens, D=512):
  q = Xq @ Wq.T + bq ; k = Xk @ Wkv.T + bkv ; v = Xv @ Wkv.T + bkv
  S = q @ k.T  (48x48), diagonal masked to -inf
  A = softmax(S / sqrt(D)) ;  O = A @ v ;  Y = O @ Wo.T + bo

Kernel strategy (per core = one b-slice, T = 7*24*48 = 8064 tokens):
  - all PE matmuls in bf16 (1 cyc/row at any moving-dim size)
  - feature-major pipeline, 7 chunks of CH=1152 tokens; chunk 0's
    transposes are split in two pieces so the first G matmul starts
    ~11us in (each HWDGE queue paces DMA starts ~2.2us apart).
      XqT, XkT via ONE merged xbar-transpose DMA each ([ch,512] ->
        [128,4,ch], chunk-major rows: partition = d%128, dc = d//128)
        on the SP HWDGE queue
      Xv via ONE strided DMA into pair-tiled [96, ch/96, 512] on the
        gpsimd SWDGE queue (so its waits never block SP prefetch)
      GT = MT-stationary @ XqT  (M = Wkv^T Wq folded on host; bias c1)
      ST = XkT-pair-stationary @ GT per 96-token pair (block-diag)
      ET = exp(scale*ST + mask)  (additive -inf mask via PE matmul;
                                  softmax normalization DEFERRED)
      Z  = per-y-tile column sums of ET: etw-slice-stationary @ ones
           -> [128,1] columns in an s-ring PSUM tile; one DVE reciprocal
      U  = Xv-stationary @ ET    (= (E @ Xv).T, feature-major)
      Y  = U-chunk-stationary @ WVOT -> token-major PSUM  (WVO = Wo Wkv)
      Y_sb = bf16(Y * (1/Z)[token] + (bo + Wo @ bkv))
      Y stored per 3 y-tiles via the gpsimd SWDGE queue (final chunk
        as [6,2,1] tiles so the tail store is short)
  - attention pair loop software-pipelined by 2 so Z/U matmuls never
    stall PE on exp's Act latency
  - v-projection bias bkv folded into output bias (softmax rows sum to 1)
  - q/k biases folded into c1 (only the Wkv^T bq term matters)
  - output stored bf16, cast to f32 on host (tolerance permits)
"""

import numpy as np

B, V, P, L, D = 8, 7, 24, 48, 512
T = V * P * L            # 8064 tokens per core
NCORES = 8
PAIR = 2 * L             # 96 tokens (2 patches) per attention tile
CH = 1152                # max tokens per pipeline chunk (12 pairs)
SUB = 384                # PSUM-bank-sized sub-block (4 pairs)
SCALE = float(D) ** -0.5

_CACHE = {}


def _ladder(tokens):
    assert tokens % SUB == 0
    out = []
    left = tokens
    while left > 0:
        c = min(CH, left)
        out.append(c)
        left -= c
    return out


def _build(tokens):
    import concourse.mybir as mybir
    import concourse.tile as tile
    from concourse import bacc

    f32 = mybir.dt.float32
    bf16 = mybir.dt.bfloat16
    AF = mybir.ActivationFunctionType

    sizes = _ladder(tokens)
    t0s = [sum(sizes[:i]) for i in range(len(sizes))]
    nchunks = len(sizes)

    nc = bacc.Bacc("TRN2", target_bir_lowering=False)

    xq_d = nc.dram_tensor("xq", [tokens, D], bf16, kind="ExternalInput")
    xk_d = nc.dram_tensor("xk", [tokens, D], bf16, kind="ExternalInput")
    xv_d = nc.dram_tensor("xv", [tokens, D], bf16, kind="ExternalInput")
    mt_d = nc.dram_tensor("mt", [D, D], bf16, kind="ExternalInput")
    wvot_d = nc.dram_tensor("wvot", [D, D], bf16, kind="ExternalInput")
    c1_d = nc.dram_tensor("c1cols", [128, 4], f32, kind="ExternalInput")
    bob_d = nc.dram_tensor("bob", [128, D], f32, kind="ExternalInput")
    mask_d = nc.dram_tensor("mask01", [PAIR, PAIR], bf16, kind="ExternalInput")
    ident96_d = nc.dram_tensor("ident96", [PAIR, PAIR], bf16, kind="ExternalInput")
    ones_d = nc.dram_tensor("ones", [128, 1], bf16, kind="ExternalInput")
    y_d = nc.dram_tensor("y", [tokens, D], bf16, kind="ExternalOutput")

    with tile.TileContext(nc) as tc:
        with (
            tc.tile_pool(name="const", bufs=1) as constp,
            tc.tile_pool(name="xt", bufs=3) as xtp,
            tc.tile_pool(name="xvp", bufs=3) as xvp,
            tc.tile_pool(name="gt", bufs=2) as gtp,
            tc.tile_pool(name="attn", bufs=2) as attnp,
            tc.tile_pool(name="outp", bufs=2) as outp,
            tc.tile_pool(name="ps_proj", bufs=2, space="PSUM") as ps_proj,
            tc.tile_pool(name="ps_s", bufs=2, space="PSUM") as ps_s,
            tc.tile_pool(name="ps_u", bufs=2, space="PSUM") as ps_u,
            tc.tile_pool(name="ps_y", bufs=2, space="PSUM") as ps_y,
        ):
            def issue_loads(ci):
                t0, ch = t0s[ci], sizes[ci]
                xqt = xtp.tile([128, 4, CH], bf16, tag="xqt")
                nc.sync.dma_start_transpose(xqt[:, :, 0:ch], xq_d[t0 : t0 + ch, :])
                xkt = xtp.tile([128, 4, CH], bf16, tag="xkt")
                nc.sync.dma_start_transpose(xkt[:, :, 0:ch], xk_d[t0 : t0 + ch, :])
                xv = xvp.tile([PAIR, CH // PAIR, D], bf16, tag="xv")
                nc.gpsimd.dma_start(
                    xv[:, 0 : ch // PAIR, :],
                    xv_d[t0 : t0 + ch, :].rearrange("(j p) d -> p j d", p=PAIR),
                )
                return xqt, xkt, xv

            # ---- warmup: consts first (mt gates the first G matmul),
            # then chunk-0 transposes in two pieces so G can start early;
            # xv rides the SWDGE queue in parallel
            mt = constp.tile([128, 4, D], bf16, tag="mt")
            wvot = constp.tile([128, 4, D], bf16, tag="wvot")
            c1 = constp.tile([128, 4], f32, tag="c1")
            bob = constp.tile([128, D], f32, tag="bob")
            mask = constp.tile([PAIR, PAIR], bf16, tag="mask")
            ident96 = constp.tile([PAIR, PAIR], bf16, tag="ident96")
            ones = constp.tile([128, 1], bf16, tag="ones")

            nc.sync.dma_start(mt, mt_d[:].rearrange("(o p) e -> p o e", p=128))
            nc.sync.dma_start(c1, c1_d[:])
            nc.sync.dma_start(mask, mask_d[:])
            nc.sync.dma_start(ident96, ident96_d[:])
            nc.sync.dma_start(ones, ones_d[:])
            xqt0 = xtp.tile([128, 4, CH], bf16, tag="xqt")
            xkt0 = xtp.tile([128, 4, CH], bf16, tag="xkt")
            xv0 = xvp.tile([PAIR, CH // PAIR, D], bf16, tag="xv")
            nc.gpsimd.dma_start(
                xv0,
                xv_d[0:CH, :].rearrange("(j p) d -> p j d", p=PAIR),
            )
            nc.sync.dma_start_transpose(xqt0[:, :, 0:SUB], xq_d[0:SUB, :])
            nc.sync.dma_start_transpose(xkt0[:, :, 0:SUB], xk_d[0:SUB, :])
            nc.sync.dma_start_transpose(xqt0[:, :, SUB:CH], xq_d[SUB:CH, :])
            nc.sync.dma_start_transpose(xkt0[:, :, SUB:CH], xk_d[SUB:CH, :])
            nc.sync.dma_start(wvot, wvot_d[:].rearrange("(o p) e -> p o e", p=128))
            nc.sync.dma_start(bob, bob_d[:])

            # ---- PE clock pre-warm: 1-column dummy matmuls gated on the
            # staggered const arrivals (~2.2us apart) keep every PE idle gap
            # under the ~3us p-state reset threshold, so the real G matmuls
            # start at full clock.
            for wsrc, wp in ((mt[:, 0, 0:128], 128), (mask, PAIR),
                             (ident96, PAIR), (ones, 1)):
                warm = ps_y.tile([128, D], f32, tag="y")
                nc.tensor.matmul(
                    warm[0:wp, 0:1], wsrc[:, 0:wp],
                    wsrc[:, 0:1], start=True, stop=True,
                )

            pend = [(xqt0, xkt0, xv0)]
            if nchunks > 1:
                pend.append(issue_loads(1))

            for c in range(nchunks):
                t0, ch = t0s[c], sizes[c]
                npair_c = ch // PAIR
                nsub_c = ch // SUB
                nyt_c = ch // 128
                xqt, xkt, xv = pend[c]
                if c + 2 < nchunks:
                    pend.append(issue_loads(c + 2))

                # ---- GT = MTC @ XqT + c1 (feature-major)
                gt = gtp.tile([128, 4, CH], bf16, tag="gt")
                for sub in range(nsub_c):
                    ss = slice(sub * SUB, (sub + 1) * SUB)
                    for ec in range(4):
                        psq = ps_proj.tile([128, SUB], f32, tag="proj")
                        for dc in range(4):
                            nc.tensor.matmul(
                                psq,
                                mt[:, dc, ec * 128 : (ec + 1) * 128],
                                xqt[:, dc, ss],
                                start=(dc == 0),
                                stop=(dc == 3),
                            )
                        nc.scalar.activation(
                            gt[:, ec, ss], psq, AF.Identity,
                            bias=c1[:, ec : ec + 1],
                        )

                # ---- attention, software-pipelined by DEPTH pairs so the
                # Z/U matmuls of pair j never stall PE on exp(j)'s latency
                u = attnp.tile([128, 4, CH], bf16, tag="u")
                etw = attnp.tile([PAIR, CH], bf16, tag="etw")
                DEPTH = 2

                def stage_s(j):
                    ls = slice(j * PAIR, (j + 1) * PAIR)
                    pss = ps_s.tile([128, PAIR], f32, tag="s")
                    for ec in range(4):
                        nc.tensor.matmul(
                            pss[:PAIR, :],
                            xkt[:, ec, ls],
                            gt[:, ec, ls],
                            start=(ec == 0),
                            stop=False,
                        )
                    nc.tensor.matmul(
                        pss[:PAIR, :], mask, ident96, start=False, stop=True
                    )
                    nc.scalar.activation(
                        etw[:, ls], pss[:PAIR, :], AF.Exp, scale=SCALE
                    )

                def stage_u(j):
                    ls = slice(j * PAIR, (j + 1) * PAIR)
                    psu = ps_u.tile([128, 4, PAIR], f32, tag="u")
                    # U[d, l] = sum_m Xv[m, d] * ET[m, l]
                    for dc in range(4):
                        nc.tensor.matmul(
                            psu[:, dc, :],
                            xv[:, j, dc * 128 : (dc + 1) * 128],
                            etw[:, ls],
                            start=True,
                            stop=True,
                        )
                    nc.vector.tensor_copy(u[:, :, ls], psu)

                for j in range(npair_c + DEPTH):
                    if j < npair_c:
                        stage_s(j)
                    if j >= DEPTH:
                        stage_u(j - DEPTH)

                # ---- Z column sums per y-tile into an s-ring tile; the DVE
                # reciprocal frees it well before the slot is reused
                zst = ps_s.tile([128, PAIR], f32, tag="s")
                for lt in range(nyt_c):
                    nc.tensor.matmul(
                        zst[:, lt : lt + 1],
                        etw[:, lt * 128 : (lt + 1) * 128],
                        ones[:PAIR, :],
                        start=True,
                        stop=True,
                    )
                zcol = attnp.tile([128, CH // 128], f32, tag="zcol")
                nc.vector.reciprocal(zcol[:, 0:nyt_c], zst[:, 0:nyt_c])

                # ---- Y = (U.T @ WVO.T) * (1/Z) + bob  (token-major),
                # stored in 3-y-tile pieces via the gpsimd queue
                ysb = outp.tile([128, CH // 128, D], bf16, tag="ysb")
                for lt in range(nyt_c):
                    lsl = slice(lt * 128, (lt + 1) * 128)
                    psy = ps_y.tile([128, D], f32, tag="y")
                    for dc in range(4):
                        nc.tensor.matmul(
                            psy,
                            u[:, dc, lsl],
                            wvot[:, dc, :],
                            start=(dc == 0),
                            stop=(dc == 3),
                        )
                    ytmp = outp.tile([128, D], f32, tag="ytmp")
                    nc.scalar.activation(
                        ytmp, psy, AF.Identity, scale=zcol[:, lt : lt + 1]
                    )
                    nc.vector.tensor_add(ysb[:, lt, :], ytmp, bob)
                    if c == nchunks - 1 and lt in (5, 7, 8):
                        # final chunk: pieces [0:6), [6:8), [8] so the very
                        # last store is one tile and the tail stays short
                        lo = 0 if lt == 5 else (6 if lt == 7 else 8)
                        nc.gpsimd.dma_start(
                            y_d[t0 + lo * 128 : t0 + (lt + 1) * 128, :].rearrange(
                                "(j p) d -> p j d", p=128
                            ),
                            ysb[:, lo : lt + 1, :],
                        )
                    elif c != nchunks - 1 and lt % 3 == 2:
                        pc = lt // 3
                        nc.gpsimd.dma_start(
                            y_d[t0 + pc * SUB : t0 + (pc + 1) * SUB, :].rearrange(
                                "(j p) d -> p j d", p=128
                            ),
                            ysb[:, pc * 3 : (pc + 1) * 3, :],
                        )

    nc.compile()
    return nc


def _host_inputs(queries, keys, values, Wq, bq, Wkv, bkv, Wo, bo, tokens):
    import ml_dtypes

    bf16 = ml_dtypes.bfloat16
    M = Wkv.astype(np.float64).T @ Wq.astype(np.float64)
    WVO = Wo.astype(np.float64) @ Wkv.astype(np.float64)
    mt = np.ascontiguousarray(M.T).astype(bf16)   # [512, 512], row d, col e
    wvot = np.ascontiguousarray(WVO.T).astype(bf16)
    c1v = Wkv.astype(np.float64).T @ bq.astype(np.float64)
    c1cols = np.ascontiguousarray(c1v.reshape(4, 128).T).astype(np.float32)
    bo_eff = bo.astype(np.float64) + Wo.astype(np.float64) @ bkv.astype(np.float64)
    bob = np.tile(bo_eff.astype(np.float32)[None, :], (128, 1))
    BIG = -1.0e30
    mneg = np.full((PAIR, PAIR), BIG, np.float32)
    blkz = np.eye(L, dtype=np.float32)  # zero where allowed
    mneg[:L, :L] = BIG * blkz
    mneg[L:, L:] = BIG * blkz
    mask01 = mneg.astype(bf16)  # additive -inf mask (diag + cross-patch)
    ident96 = np.eye(PAIR, dtype=np.float32).astype(bf16)
    ones = np.ones((128, 1), np.float32).astype(bf16)

    shared = dict(
        mt=mt, wvot=wvot, c1cols=c1cols, bob=bob,
        mask01=mask01, ident96=ident96, ones=ones,
    )
    in_maps = []
    for core in range(NCORES):
        m = dict(shared)
        m["xq"] = queries[core].reshape(-1, D)[:tokens].astype(bf16)
        m["xk"] = keys[core].reshape(-1, D)[:tokens].astype(bf16)
        m["xv"] = values[core].reshape(-1, D)[:tokens].astype(bf16)
        in_maps.append(m)
    return in_maps


def kernel(queries, keys, values, Wq, bq, Wkv, bkv, Wo, bo, _tokens=T, _trace=False):
    queries = np.asarray(queries)
    keys = np.asarray(keys)
    values = np.asarray(values)
    from concourse.bass_utils import run_bass_kernel_spmd

    key = _tokens
    if key not in _CACHE:
        _CACHE[key] = _build(_tokens)
    nc = _CACHE[key]

    in_maps = _host_inputs(
        queries, keys, values,
        np.asarray(Wq), np.asarray(bq), np.asarray(Wkv), np.asarray(bkv),
        np.asarray(Wo), np.asarray(bo), _tokens,
    )
    res = run_bass_kernel_spmd(
        nc, in_maps, core_ids=list(range(NCORES)), trace=_trace,
    )
    outs = [np.asarray(res.results[i]["y"], dtype=np.float32) for i in range(NCORES)]
    if _tokens == T:
        full = np.stack([o.reshape(V, P, L, D) for o in outs], axis=0)
    else:
        full = np.stack(outs, axis=0)
    if _trace:
        return full, res
    return full
